# revision 11
# baseline (speedup 1.0000x reference)
"""Trainium2 Bass kernel for the differentiable LogicLayer forward pass.

Math (per output neuron j with a = x[:, idx_a[j]], b = x[:, idx_b[j]]):
    w      = softmax(weights[j])          # [14]
    coeffs = w @ OP_COEFFS                # [4] -> c0, ca, cb, cab
    out[:, j] = c0 + ca*a + cb*b + cab*a*b

Sharding: data-parallel over batch across 8 NeuronCores (1024 rows each);
weights / indices replicated.  Per core the kernel works feature-major:
partition p holds output neuron q = ci*1024 + s*128 + p (in idx_a-SORTED
order), the free dim holds the 1024-sample batch shard.

The a-operand is NOT DMA-gathered: neurons are sorted by idx_a on the
host, so each 128-neuron slice reads a narrow feature range and
a = onehot^T @ xT becomes one (occasionally two) 128x128 PE matmuls
against the SBUF-resident xT (fp16, 8 MiB), accumulating in PSUM.  Only
the b-operand uses the SWDGE dma_gather (1024 idxs/call, the HW cap),
halving the Q7 descriptor-generation serial cost that bounds the
all-SWDGE variant.  PSUM matmuls are 512 columns per bank, two per
slice into one 2-bank tile.

Everything else is fp16 (x is uniform[0,1), 2e-2 budget >> fp16
rounding).  Softmax coefficient collapse runs on the host.  Per slice:
    u = cab*a + cb        (ACT from PSUM)
    w = u*b               (DVE tensor_tensor, 2x fp16)
    o = (ca*a + c0) + w   - even slices: ACT v then DVE tt add;
                            odd slices: fused DVE affine_then_add
The store uses a tile-contiguous DRAM scratch layout [ci, p, s, b]
(16 KiB DMA lines); the host unscrambles, unpermutes, and upcasts.

The program depends on idx_a (matmul block offsets), so it is built and
compiled on the first kernel() call.
"""

import sys

import numpy as np

try:  # the axon sitecustomize usually provides concourse already
    import concourse  # noqa: F401
except ImportError:  # pragma: no cover
    sys.path.insert(0, "/opt/trn_rl_repo")

import concourse.bacc as bacc
import concourse.mybir as mybir
import concourse.tile as tile
from concourse.bass_utils import run_bass_kernel_spmd
from concourse.library_config import mlp as mlp_library

F32 = mybir.dt.float32
F16 = mybir.dt.float16
I16 = mybir.dt.int16

NCORES = 8
BATCH, IN_DIM, OUT_DIM, NOPS = 8192, 4096, 16384, 14
B = BATCH // NCORES            # 1024 batch rows per core
NJC = 1024                     # output neurons per chunk (SWDGE 1024-idx cap)
NCH = OUT_DIM // NJC           # 16 chunks
SL = NJC // 128                # 8 partition-slices per chunk
NSLICE = OUT_DIM // 128        # 128 slices / coefficient columns
NBLK = IN_DIM // 128           # 32 feature blocks
HB = B // 2                    # 512-column PSUM bank halves

CAB_CLAMP = 1e-3               # |cab| floor for the S-layout division


def _slice_kind(t):
    """R/Q/S layout per slice: 60% R, 20% Q, 20% S (engine balance)."""
    r = t % 5
    return "R" if r < 3 else ("Q" if r == 3 else "S")


_OP_COEFFS = np.array([
    [0,  0,  0,  1],
    [0,  1,  0, -1],
    [0,  1,  0,  0],
    [0,  0,  1, -1],
    [0,  0,  1,  0],
    [0,  1,  1, -2],
    [0,  1,  1, -1],
    [1, -1, -1,  1],
    [1, -1, -1,  2],
    [1,  0, -1,  0],
    [1,  0, -1,  1],
    [1, -1,  0,  0],
    [1, -1,  0,  1],
    [1,  0,  0, -1],
], dtype=np.float32)


def _plan_slices(sorted_ia):
    """Per 128-neuron slice: list of feature blocks it spans (sorted)."""
    plans = []
    for m in range(NSLICE):
        v = sorted_ia[m * 128:(m + 1) * 128]
        plans.append(list(range(int(v[0]) // 128, int(v[-1]) // 128 + 1)))
    return plans


def build_program(slice_blocks):
    """Build + compile the per-core Bass program.

    slice_blocks[m] is the list of xT feature blocks slice m's one-hot
    matmuls read; the one-hot lhsT tiles are streamed from DRAM in the
    same flat order.
    """
    n_mm = sum(len(bl) for bl in slice_blocks)

    nc = bacc.Bacc("TRN2", target_bir_lowering=False, debug=False,
                   num_devices=NCORES)

    # xT natural [4096, B] for the SWDGE b-gather
    xt = nc.dram_tensor("xt", [IN_DIM, B], F16, kind="ExternalInput")
    # xT wrapped [128, NBLK, B]: partition k, block c -> feature c*128+k
    xtw = nc.dram_tensor("xtw", [128, NBLK, B], F16, kind="ExternalInput")
    # one-hot lhsT tiles, SBUF-wrapped [128(k), n_mm, 128(m)]
    oh = nc.dram_tensor("oh", [128, n_mm, 128], F16, kind="ExternalInput")
    cf32 = nc.dram_tensor("cf32", [128, 5, NSLICE], F32, kind="ExternalInput")
    idxb = nc.dram_tensor("idxb", [128, OUT_DIM // 16], I16,
                          kind="ExternalInput")
    # tile-contiguous scratch layout: [ci, p, s, b]; host unscrambles.
    out = nc.dram_tensor("out", [NCH, 128, SL, B], F16, kind="ExternalOutput")
    out_r = out.ap()

    mult = mybir.AluOpType.mult
    add = mybir.AluOpType.add
    ident = mybir.ActivationFunctionType.Identity

    with tile.TileContext(nc) as tc:
        nc.gpsimd.load_library(mlp_library)
        with (
            tc.tile_pool(name="const", bufs=1) as cpool,
        ):
            ib_sb = cpool.tile([128, OUT_DIM // 16], I16)
            nc.sync.dma_start(ib_sb[:], idxb.ap())
            c32 = cpool.tile([128, 5, NSLICE], F32)
            nc.sync.dma_start(c32[:], cf32.ap())
            c0_32, ca_32 = c32[:, 0], c32[:, 1]
            cb_32, cab_32 = c32[:, 2], c32[:, 3]
            cad_32 = c32[:, 4]
            # per-block xT tiles and per-chunk one-hot tiles so early
            # slices only wait on their own data, not the full 12 MiB
            xts = cpool.tile([128, NBLK, B], F16)
            for blk in range(NBLK):
                nc.sync.dma_start(xts[:, blk], xtw.ap()[:, blk])
            # chunk mm ranges
            mm_starts = [0]
            for ci in range(NCH):
                mm_starts.append(mm_starts[-1] + sum(
                    len(slice_blocks[ci * SL + s_]) for s_ in range(SL)))
            oh_sb = cpool.tile([128, n_mm, 128], F16)
            for ci in range(NCH):
                lo, hi = mm_starts[ci], mm_starts[ci + 1]
                nc.sync.dma_start(oh_sb[:, lo:hi], oh.ap()[:, lo:hi])

            with (
                tc.tile_pool(name="gb", bufs=2) as bpool,
                tc.tile_pool(name="ps", bufs=4, space="PSUM") as pspool,
                tc.tile_pool(name="go", bufs=2) as opool,
                tc.tile_pool(name="uv", bufs=4) as uvpool,
            ):
                w16 = NJC // 16  # idx columns per chunk
                mm_i = 0
                for ci in range(NCH):
                    bt = bpool.tile([128, SL, B], F16)
                    nc.gpsimd.dma_gather(
                        bt[:], xt.ap(), ib_sb[:, ci * w16:(ci + 1) * w16],
                        NJC, NJC, B)
                    ot = opool.tile([128, SL, B], F16)
                    for s in range(SL):
                        t = ci * SL + s
                        blocks = slice_blocks[t]
                        ps = pspool.tile([128, B], F32)
                        mm0 = mm_i
                        mm_i += len(blocks)
                        for h in range(2):
                            for bi, blk in enumerate(blocks):
                                nc.tensor.matmul(
                                    ps[:, h * HB:(h + 1) * HB],
                                    lhsT=oh_sb[:, mm0 + bi],
                                    rhs=xts[:, blk, h * HB:(h + 1) * HB],
                                    start=bi == 0,
                                    stop=bi == len(blocks) - 1)
                        b_s = bt[:, s]
                        kind = _slice_kind(t)
                        if kind == "S":
                            # psum = cab'*a (scaled one-hot).
                            # w = (psum + cb)*b  (DVE stt)
                            w = uvpool.tile([128, B], F16, tag="u")
                            nc.vector.scalar_tensor_tensor(
                                w[:], ps[:], cb_32[:, t:t + 1], b_s,
                                op0=add, op1=mult)
                            # o = (ca/cab')*psum + c0 + w  (DVE ata)
                            nc.vector.affine_then_add(
                                ot[:, s], ps[:], w[:],
                                scale=cad_32[:, t:t + 1],
                                bias=c0_32[:, t:t + 1])
                            continue
                        u = uvpool.tile([128, B], F16, tag="u")
                        # u = cab*a + cb  (ACT from PSUM)
                        nc.scalar.activation(u[:], ps[:], ident,
                                             bias=cb_32[:, t:t + 1],
                                             scale=cab_32[:, t:t + 1])
                        # w = u*b  (DVE tensor_tensor, 2x fp16)
                        nc.vector.tensor_tensor(u[:], u[:], b_s, op=mult)
                        if kind == "R":
                            # R: v on ACT, final add on DVE (tt, 2x fp16)
                            v = uvpool.tile([128, B], F16, tag="v")
                            nc.scalar.activation(v[:], ps[:], ident,
                                                 bias=c0_32[:, t:t + 1],
                                                 scale=ca_32[:, t:t + 1])
                            nc.vector.tensor_tensor(ot[:, s], u[:], v[:],
                                                    op=add)
                        else:
                            # Q: fused (a*ca + c0) + w on DVE
                            nc.vector.affine_then_add(
                                ot[:, s], ps[:], u[:],
                                scale=ca_32[:, t:t + 1],
                                bias=c0_32[:, t:t + 1])
                    nc.sync.dma_start(out_r[ci], ot[:])
                assert mm_i == n_mm

    nc.compile()
    return nc


_PROGRAM = None
_PLAN_KEY = None


def _coeff_tensors(weights):
    """softmax(weights) @ OP_COEFFS -> [128, 5, NSLICE] f32 device layout.

    Plane 4 is ca/cab' with cab' = cab clamped away from 0 (S-layout);
    the clamp only perturbs the cab*a*b term by <= CAB_CLAMP.  Returns
    (planes, cab_clamped[OUT_DIM]) — the latter scales S one-hots.
    """
    w = weights.astype(np.float32)
    e = np.exp(w - w.max(axis=1, keepdims=True))
    sm = e / e.sum(axis=1, keepdims=True)
    coef = sm @ _OP_COEFFS                      # [OUT_DIM, 4]
    cab = coef[:, 3]
    cabc = np.where(np.abs(cab) < CAB_CLAMP,
                    np.where(cab < 0, -CAB_CLAMP, CAB_CLAMP), cab)
    cadiv = coef[:, 1] / cabc
    full = np.concatenate([coef, cadiv[:, None]], axis=1)  # [OUT_DIM, 5]
    c = full.reshape(NSLICE, 128, 5).transpose(1, 2, 0)
    return np.ascontiguousarray(c, dtype=np.float32), cabc


def _wrap_idx(idx):
    """[OUT_DIM] int -> SWDGE-wrapped int16 [128, OUT_DIM//16]."""
    i16 = idx.astype(np.int16).reshape(NCH, NJC // 16, 16)
    w = i16.transpose(2, 0, 1).reshape(16, NCH * (NJC // 16))
    return np.ascontiguousarray(np.tile(w, (8, 1)))


def _build_onehots(sorted_ia, slice_blocks, cabc):
    """[128(k), n_mm, 128(m)] fp16 lhsT tiles: oh[k, i, m] = g iff
    sorted_ia[slice_of(i)*128+m] == blk_of(i)*128 + k, where g = 1 for
    R/Q slices and cab'[neuron] for S slices (psum arrives pre-scaled)."""
    tiles = []
    for m, blocks in enumerate(slice_blocks):
        v = sorted_ia[m * 128:(m + 1) * 128]
        if _slice_kind(m) == "S":
            g = cabc[m * 128:(m + 1) * 128].astype(np.float16)
        else:
            g = np.ones(128, dtype=np.float16)
        for blk in blocks:
            t = np.zeros((128, 128), dtype=np.float16)
            k = v - blk * 128
            sel = (k >= 0) & (k < 128)
            cols = np.nonzero(sel)[0]
            t[k[sel], cols] = g[cols]
            tiles.append(t)
    return np.ascontiguousarray(np.stack(tiles).transpose(1, 0, 2))


def _get_program_and_plan(idx_a):
    global _PROGRAM, _PLAN_KEY
    key = idx_a.tobytes()
    if _PROGRAM is None or _PLAN_KEY != key:
        perm = np.argsort(idx_a, kind="stable")
        slice_blocks = _plan_slices(idx_a[perm])
        _PROGRAM = (build_program(slice_blocks), perm, slice_blocks)
        _PLAN_KEY = key
    return _PROGRAM


def prepare_in_maps(x, weights, idx_a, idx_b, perm, slice_blocks):
    x = np.asarray(x, dtype=np.float32)
    weights = np.asarray(weights, dtype=np.float32)
    sorted_ia = idx_a[perm]
    cf32, cabc = _coeff_tensors(weights[perm])
    ib = _wrap_idx(np.asarray(idx_b)[perm])
    ohs = _build_onehots(sorted_ia, slice_blocks, cabc)

    in_maps = []
    for c in range(NCORES):
        xt = np.ascontiguousarray(x[c * B:(c + 1) * B].T).astype(np.float16)
        xtw = np.ascontiguousarray(
            xt.reshape(NBLK, 128, B).transpose(1, 0, 2))
        in_maps.append({"xt": xt, "xtw": xtw, "oh": ohs, "cf32": cf32,
                        "idxb": ib})
    return in_maps


def assemble_output(results, perm):
    out = np.empty((BATCH, OUT_DIM), dtype=np.float32)
    for c in range(NCORES):
        # scratch [NCH, 128, SL, B] -> [B, q] with q = ci*NJC + s*128 + p,
        # then un-permute: out[:, perm[q]] = sorted_out[:, q]
        scr = results[c]["out"]
        srt = scr.transpose(3, 0, 2, 1).reshape(B, OUT_DIM)
        out[c * B:(c + 1) * B, perm] = srt.astype(np.float32)
    return out


def kernel(x, weights, idx_a, idx_b):
    idx_a = np.asarray(idx_a)
    nc, perm, slice_blocks = _get_program_and_plan(idx_a)
    in_maps = prepare_in_maps(x, weights, idx_a, np.asarray(idx_b),
                              perm, slice_blocks)
    res = run_bass_kernel_spmd(nc, in_maps, list(range(NCORES)))
    return assemble_output(res.results, perm)
